# revision 11
# baseline (speedup 1.0000x reference)
"""Trainium2 Bass kernel for nn_IsocortexSubstrate.

The reference network is three chained single-step SSM layers, each applied to
a fresh (all-zero) hidden state.  With h_prev = 0 the recurrent term
h_prev @ A.T vanishes, so layer k reduces to

    y_k = x_k * dot(B_k, C_k)          (per element)
    spikes_k = (sigmoid(y_k) > 0.5) = (y_k > 0)

Since spikes are in {0, 1}, chaining three layers collapses to a single
elementwise gate on the input:

    out = x * g,   g = [min(s1, s2, s3) > 0],  s_k = dot(B_k, C_k)

Two observations make the device work almost free:

  * spikes are binary, so we ship them as uint8 (exact in 1 byte) instead of
    f32 -- 4x less HBM traffic;
  * the output is either a byte-identical copy of the input (g = 1) or all
    zeros (g = 0), so no elementwise pass is needed at all: *predicated*
    DMAs (dma_start(cond=...), skip-if-false, semaphore fires either way)
    cover both cases and exactly one set of them executes.

Device schedule (raw Bass):

  sync engine:   two 256 KiB spike-load chunks, then (once the gate register
                 is loaded) predicated copy-stores of partitions 0-63.  The
                 stores ride the SAME HWDGE ring as the loads: per-ring FIFO
                 on each SDMA engine means the store descriptors drain right
                 behind the load descriptors for the same partitions, so no
                 load-completion semaphore is needed on this engine.
  scalar engine: the tiny 384 B B/C load first (its ring is idle early),
                 then predicated copy-stores of partitions 64-127 (these
                 wait on the load-chunk semaphores since they use the other
                 ring), then the predicated zeros-store for the whole slab
                 (cond = not g).  Two rings stream stores concurrently.
  vector engine: zeros memset, then after the B/C load three small ops:
                 products [1,48], per-layer dot sums [1,3], min [1,1].
  sequencers:    sync and scalar each reg_load the f32 min and compare > 0
                 in the register file (sign of the f32 bit pattern), so the
                 DVE never touches the bulk data at all.

The gate chain (B/C DMA + 3 small vector ops + register load) hides under
the 512 KiB spike load stream.
"""

import sys

sys.path.insert(0, "/opt/trn_rl_repo")

import numpy as np

N_CORES = 8
BATCH = 4096
WIDTH = 1024
ROWS = BATCH // N_CORES          # 512 spike-rows per core
P = 128                          # SBUF partitions
COLS = ROWS * WIDTH // 4 // P    # 1024 int32 per partition (4 spikes/word)
HALF = COLS // 2                 # load chunk: [128, 512] i32 = 256 KiB
HP = P // 2                      # store chunk: [64, COLS] partitions

_cache = {}


def _build():
    import contextlib

    import concourse.bass as bass
    import concourse.mybir as mybir

    f32 = mybir.dt.float32
    i32 = mybir.dt.int32
    is_gt = mybir.AluOpType.is_gt
    is_le = mybir.AluOpType.is_le
    amin = mybir.AluOpType.min
    add = mybir.AluOpType.add

    nc = bass.Bass("TRN2", target_bir_lowering=False, debug=False,
                   enable_asserts=False, num_devices=N_CORES)
    x_in = nc.dram_tensor("x", [P, COLS], i32, kind="ExternalInput")
    bc_in = nc.dram_tensor("bc", [1, 96], f32, kind="ExternalInput")
    y_out = nc.dram_tensor("y", [P, COLS], i32, kind="ExternalOutput")

    with contextlib.ExitStack() as stack:
        sem = lambda name: stack.enter_context(nc.semaphore(name))
        bc_sem = sem("bc_in")
        x_sems = [sem("x_in0"), sem("x_in1")]
        g_sem = sem("g")
        so_sem = sem("so")

        sb = stack.enter_context
        bcT = sb(nc.sbuf_tensor("bcT", [1, 96], f32))
        prod = sb(nc.sbuf_tensor("prod", [1, 48], f32))
        s3 = sb(nc.sbuf_tensor("s3", [1, 3], f32))
        smin = sb(nc.sbuf_tensor("smin", [1, 1], f32))
        gg = sb(nc.sbuf_tensor("gg", [1, 2], i32))
        xt = sb(nc.sbuf_tensor("xt", [P, COLS], i32))
        zt = sb(nc.sbuf_tensor("zt", [P, COLS], i32))

        # prod viewed as [1, 3, 16] so tensor_reduce(X) sums each layer's
        # 16 products into one dot
        pb = prod[:]
        prod3 = bass.AP(tensor=pb.tensor, offset=pb.offset,
                        ap=[list(pb.ap[0]), [16, 3], [1, 16]])

        xa = x_in.ap()
        ya = y_out.ap()

        def load_gate(eng, name, col):
            """load the precomputed 0/1 int32 gate flag into a register"""
            reg = eng.alloc_register(name)
            eng.reg_load(reg, gg[0:1, col:col + 1])
            # snap declares the [0,1] range without emitting a SeqAssert
            # (walrus cannot encode SeqAssert)
            return eng.snap(reg, donate=True, min_val=0, max_val=1)

        with nc.Block() as block:

            @block.sync
            def _(sync):
                sync.dma_start(out=xt[:, 0:HALF], in_=xa[:, 0:HALF]).then_inc(
                    x_sems[0], 16)
                sync.dma_start(out=xt[:, HALF:COLS], in_=xa[:, HALF:COLS]
                               ).then_inc(x_sems[1], 16)
                sync.wait_ge(g_sem, 1)
                gval = load_gate(sync, "greg", 0)
                # partitions 0-63: same ring as the loads -> FIFO-ordered
                sync.dma_start(out=ya[0:HP, 0:HALF], in_=xt[0:HP, 0:HALF],
                               cond=gval).then_inc(so_sem, 16)
                sync.dma_start(out=ya[0:HP, HALF:COLS],
                               in_=xt[0:HP, HALF:COLS],
                               cond=gval).then_inc(so_sem, 16)
                sync.wait_ge(so_sem, 16 * 5)
                sync.wait_ge(x_sems[0], 16)
                sync.wait_ge(x_sems[1], 16)

            @block.scalar
            def _(scalar):
                scalar.dma_start(out=bcT[:], in_=bc_in.ap()).then_inc(
                    bc_sem, 16)
                scalar.wait_ge(g_sem, 1)
                gval = load_gate(scalar, "greg2", 0)
                gnval = load_gate(scalar, "gnreg", 1)
                # partitions 64-127: other ring, needs load completion
                scalar.wait_ge(x_sems[0], 16)
                scalar.dma_start(out=ya[HP:P, 0:HALF], in_=xt[HP:P, 0:HALF],
                                 cond=gval).then_inc(so_sem, 16)
                scalar.wait_ge(x_sems[1], 16)
                scalar.dma_start(out=ya[HP:P, HALF:COLS],
                                 in_=xt[HP:P, HALF:COLS],
                                 cond=gval).then_inc(so_sem, 16)
                scalar.dma_start(out=ya[:], in_=zt[:], cond=gnval).then_inc(
                    so_sem, 16)

            @block.vector
            def _(vector):
                vector.memset(zt[:], 0)
                vector.wait_ge(bc_sem, 16)
                vector.tensor_mul(prod[:], bcT[:, 0:48], bcT[:, 48:96])
                vector.drain()
                vector.tensor_reduce(s3[:], prod3, axis=mybir.AxisListType.X,
                                     op=add)
                vector.drain()
                vector.tensor_reduce(smin[:], s3[:], axis=mybir.AxisListType.X,
                                     op=amin)
                vector.drain()
                vector.tensor_scalar(out=gg[0:1, 0:1], in0=smin[:],
                                     scalar1=0.0, scalar2=None, op0=is_gt)
                vector.tensor_scalar(out=gg[0:1, 1:2], in0=smin[:],
                                     scalar1=0.0, scalar2=None, op0=is_le)
                vector.drain()
                vector.sem_inc(g_sem, 1)

    return nc


def _get_nc():
    if "nc" not in _cache:
        _cache["nc"] = _build()
    return _cache["nc"]


def _prep_in_maps(
    incoming_spikes,
    B_sensory, C_sensory, B_association, C_association,
    B_executive, C_executive,
):
    x = np.asarray(incoming_spikes)
    # spikes are {0,1}; ship them as one byte each ((x>0) matches the
    # sigmoid(y)>0.5 threshold for any non-negative input)
    xb = np.ascontiguousarray((x > 0).astype(np.uint8))
    xw = xb.view(np.int32).reshape(N_CORES, P, COLS)
    bc = np.concatenate(
        [
            np.asarray(B_sensory, dtype=np.float32).reshape(16),
            np.asarray(B_association, dtype=np.float32).reshape(16),
            np.asarray(B_executive, dtype=np.float32).reshape(16),
            np.asarray(C_sensory, dtype=np.float32).reshape(16),
            np.asarray(C_association, dtype=np.float32).reshape(16),
            np.asarray(C_executive, dtype=np.float32).reshape(16),
        ]
    ).reshape(1, 96)
    return [{"x": xw[i], "bc": bc} for i in range(N_CORES)]


def kernel(
    incoming_spikes,
    A_sensory, B_sensory, C_sensory,
    A_association, B_association, C_association,
    A_executive, B_executive, C_executive,
):
    from concourse.bass_utils import run_bass_kernel_spmd

    nc = _get_nc()
    in_maps = _prep_in_maps(
        incoming_spikes,
        B_sensory, C_sensory, B_association, C_association,
        B_executive, C_executive,
    )
    res = run_bass_kernel_spmd(nc, in_maps, list(range(N_CORES)))
    out = np.concatenate(
        [
            np.ascontiguousarray(res.results[i]["y"])
            .view(np.uint8)
            .reshape(ROWS, WIDTH)
            for i in range(N_CORES)
        ],
        axis=0,
    )
    return out.astype(np.float32)


# revision 13
# speedup vs baseline: 1.1689x; 1.1689x over previous
"""Trainium2 Bass kernel for nn_IsocortexSubstrate.

The reference network is three chained single-step SSM layers, each applied to
a fresh (all-zero) hidden state.  With h_prev = 0 the recurrent term
h_prev @ A.T vanishes, so layer k reduces to

    y_k = x_k * dot(B_k, C_k)          (per element)
    spikes_k = (sigmoid(y_k) > 0.5) = (y_k > 0)

Since spikes are in {0, 1}, chaining three layers collapses to a single
elementwise gate on the input:

    out = x * g,   g = [min(s1, s2, s3) > 0],  s_k = dot(B_k, C_k)

Two observations make the device work almost free:

  * spikes are binary, so the wire format is 1 bit per spike (packbits on
    the host, exact): 64 KiB per core each way instead of 2 MiB of f32;
  * the output is either a byte-identical copy of the input (g = 1) or all
    zeros (g = 0), so no elementwise pass is needed at all: *predicated*
    DMAs (dma_start(cond=...), skip-if-false, semaphore fires either way)
    cover both cases and exactly one of them executes.

Device schedule (raw Bass):

  sync engine:   tiny 384 B B/C load, the 64 KiB packed-spike load, then
                 (once the gate register is loaded) the predicated
                 copy-store.  All three ride ONE HWDGE ring: per-ring FIFO
                 on each SDMA engine means the store descriptors drain right
                 behind the load descriptors for the same partitions, so the
                 store needs no load-completion semaphore.
  vector engine: zeros memset, then after the B/C load: products [1,48],
                 per-layer dot sums [1,3], min [1,1], gate flag int32 [1,1].
  scalar engine: predicated zeros-store (cond = not g, derived from g with
                 one register ALU op).

The gate chain (B/C DMA + 4 small vector ops + register load) is the
critical path; everything else hides under or behind it.
"""

import sys

sys.path.insert(0, "/opt/trn_rl_repo")

import numpy as np

N_CORES = 8
BATCH = 4096
WIDTH = 1024
ROWS = BATCH // N_CORES          # 512 spike-rows per core
P = 128                          # SBUF partitions
COLS = ROWS * WIDTH // 32 // P   # 128 int32 per partition (32 spikes/word)

_cache = {}


def _build():
    import contextlib

    import concourse.bass as bass
    import concourse.mybir as mybir

    f32 = mybir.dt.float32
    i32 = mybir.dt.int32
    is_gt = mybir.AluOpType.is_gt
    is_equal = mybir.AluOpType.is_equal
    amin = mybir.AluOpType.min
    add = mybir.AluOpType.add

    nc = bass.Bass("TRN2", target_bir_lowering=False, debug=False,
                   enable_asserts=False, num_devices=N_CORES)
    x_in = nc.dram_tensor("x", [P, COLS], i32, kind="ExternalInput")
    bc_in = nc.dram_tensor("bc", [1, 96], f32, kind="ExternalInput")
    y_out = nc.dram_tensor("y", [P, COLS], i32, kind="ExternalOutput")

    with contextlib.ExitStack() as stack:
        sem = lambda name: stack.enter_context(nc.semaphore(name))
        bc_sem = sem("bc_in")
        x_sem = sem("x_in")
        g_sem = sem("g")
        so_sem = sem("so")

        sb = stack.enter_context
        bcT = sb(nc.sbuf_tensor("bcT", [1, 96], f32))
        prod = sb(nc.sbuf_tensor("prod", [1, 48], f32))
        s3 = sb(nc.sbuf_tensor("s3", [1, 3], f32))
        smin = sb(nc.sbuf_tensor("smin", [1, 1], f32))
        gg = sb(nc.sbuf_tensor("gg", [1, 1], i32))
        xt = sb(nc.sbuf_tensor("xt", [P, COLS], i32))
        zt = sb(nc.sbuf_tensor("zt", [P, COLS], i32))

        # prod viewed as [1, 3, 16] so tensor_reduce(X) sums each layer's
        # 16 products into one dot
        pb = prod[:]
        prod3 = bass.AP(tensor=pb.tensor, offset=pb.offset,
                        ap=[list(pb.ap[0]), [16, 3], [1, 16]])

        with nc.Block() as block:

            @block.sync
            def _(sync):
                sync.dma_start(out=bcT[:], in_=bc_in.ap()).then_inc(bc_sem, 16)
                sync.dma_start(out=xt[:], in_=x_in.ap()).then_inc(x_sem, 16)
                sync.wait_ge(g_sem, 1)
                greg = sync.alloc_register("greg")
                sync.reg_load(greg, gg[0:1, 0:1])
                # snap declares the [0,1] range without emitting a SeqAssert
                # (walrus cannot encode SeqAssert)
                gval = sync.snap(greg, donate=True, min_val=0, max_val=1)
                sync.dma_start(out=y_out.ap(), in_=xt[:], cond=gval).then_inc(
                    so_sem, 16)
                sync.wait_ge(so_sem, 32)
                sync.wait_ge(x_sem, 16)

            @block.scalar
            def _(scalar):
                scalar.wait_ge(g_sem, 1)
                gnreg = scalar.alloc_register("gnreg")
                scalar.reg_load(gnreg, gg[0:1, 0:1])
                scalar.reg_alu(gnreg, gnreg, 0, is_equal)   # not g
                gnval = scalar.snap(gnreg, donate=True, min_val=0, max_val=1)
                scalar.dma_start(out=y_out.ap(), in_=zt[:], cond=gnval
                                 ).then_inc(so_sem, 16)

            @block.vector
            def _(vector):
                vector.memset(zt[:], 0)
                vector.wait_ge(bc_sem, 16)
                # explicit drain between dependent DVE ops: same-engine RAW
                # is NOT covered by the automatic pipeline flush (measured:
                # without these the gate reads stale data on ~half the cores)
                vector.tensor_mul(prod[:], bcT[:, 0:48], bcT[:, 48:96])
                vector.drain()
                vector.tensor_reduce(s3[:], prod3, axis=mybir.AxisListType.X,
                                     op=add)
                vector.drain()
                vector.tensor_reduce(smin[:], s3[:], axis=mybir.AxisListType.X,
                                     op=amin)
                vector.drain()
                vector.tensor_scalar(out=gg[:], in0=smin[:],
                                     scalar1=0.0, scalar2=None, op0=is_gt)
                vector.drain()
                vector.sem_inc(g_sem, 1)

    return nc


def _get_nc():
    if "nc" not in _cache:
        _cache["nc"] = _build()
    return _cache["nc"]


def _prep_in_maps(
    incoming_spikes,
    B_sensory, C_sensory, B_association, C_association,
    B_executive, C_executive,
):
    x = np.asarray(incoming_spikes)
    # spikes are {0,1}; pack them 1 bit each ((x>0) matches the
    # sigmoid(y)>0.5 threshold for any non-negative input)
    xb = np.packbits(np.asarray(x > 0), axis=1)
    xw = np.ascontiguousarray(xb).view(np.int32).reshape(N_CORES, P, COLS)
    bc = np.concatenate(
        [
            np.asarray(B_sensory, dtype=np.float32).reshape(16),
            np.asarray(B_association, dtype=np.float32).reshape(16),
            np.asarray(B_executive, dtype=np.float32).reshape(16),
            np.asarray(C_sensory, dtype=np.float32).reshape(16),
            np.asarray(C_association, dtype=np.float32).reshape(16),
            np.asarray(C_executive, dtype=np.float32).reshape(16),
        ]
    ).reshape(1, 96)
    return [{"x": xw[i], "bc": bc} for i in range(N_CORES)]


def kernel(
    incoming_spikes,
    A_sensory, B_sensory, C_sensory,
    A_association, B_association, C_association,
    A_executive, B_executive, C_executive,
):
    from concourse.bass_utils import run_bass_kernel_spmd

    nc = _get_nc()
    in_maps = _prep_in_maps(
        incoming_spikes,
        B_sensory, C_sensory, B_association, C_association,
        B_executive, C_executive,
    )
    res = run_bass_kernel_spmd(nc, in_maps, list(range(N_CORES)))
    packed = np.concatenate(
        [
            np.ascontiguousarray(res.results[i]["y"])
            .view(np.uint8)
            .reshape(ROWS, WIDTH // 8)
            for i in range(N_CORES)
        ],
        axis=0,
    )
    return np.unpackbits(packed, axis=1).astype(np.float32)


# revision 14
# speedup vs baseline: 1.2448x; 1.0649x over previous
"""Trainium2 Bass kernel for nn_IsocortexSubstrate.

The reference network is three chained single-step SSM layers, each applied to
a fresh (all-zero) hidden state.  With h_prev = 0 the recurrent term
h_prev @ A.T vanishes, so layer k reduces to

    y_k = x_k * dot(B_k, C_k)          (per element)
    spikes_k = (sigmoid(y_k) > 0.5) = (y_k > 0)

Since spikes are in {0, 1}, chaining three layers collapses to a single
elementwise gate on the input:

    out = x * g,   g = [min(s1, s2, s3) > 0],  s_k = dot(B_k, C_k)

Three observations make the device work almost free:

  * spikes are binary, so the wire format is 1 bit per spike (packbits on
    the host, exact): 64 KiB per core each way instead of 2 MiB of f32;
  * the output is either a byte-identical copy of the input (g = 1) or all
    zeros (g = 0), so no elementwise pass is needed at all: *predicated*
    DMAs (dma_start(cond=...), skip-if-false, semaphore fires either way)
    cover both cases and exactly one of them executes;
  * the 384 B of B/C parameters ride in the tail of the spike slab
    (partition 0, cols 256+), so ONE 64-partition DMA delivers everything.
    64 partitions map to the even SDMA engines only -- engine 15, which
    starts ~1.5 us late on some runs and would gate the completion
    semaphore, never touches the critical path.

Device schedule (raw Bass):

  sync engine:   the [64,352] f32 slab load, then (once the gate register is
                 loaded) the predicated copy-store of cols 0-255.  Both ride
                 ONE HWDGE ring: per-ring FIFO on each SDMA engine means the
                 store descriptors drain right behind the load descriptors
                 for the same partitions, so the store needs no
                 load-completion semaphore and streams immediately.
  vector engine: zeros memset, then after the slab lands: products [1,48],
                 per-layer dot sums [1,3], min [1,1], gate flag int32 [1,1].
                 Explicit drain() between dependent DVE ops is required --
                 same-engine RAW is NOT covered by the automatic pipeline
                 flush (without them the gate reads stale data on ~half the
                 cores).
  scalar engine: predicated zeros-store (cond = not g, one register ALU).

The spike payload is dtype-opaque: it moves DMA-only (raw bytes), so the
slab is typed f32 for the benefit of the B/C tail the DVE reads.
"""

import sys

sys.path.insert(0, "/opt/trn_rl_repo")

import numpy as np

N_CORES = 8
BATCH = 4096
WIDTH = 1024
ROWS = BATCH // N_CORES          # 512 spike-rows per core
P = 64                           # partitions used (even SDMA engines only)
XCOLS = ROWS * WIDTH // 32 // P  # 256 f32 of packed spikes per partition
BCOLS = 96                       # B/C tail on partition 0
COLS = XCOLS + BCOLS             # 352

_cache = {}


def _build():
    import contextlib

    import concourse.bass as bass
    import concourse.mybir as mybir

    f32 = mybir.dt.float32
    i32 = mybir.dt.int32
    is_gt = mybir.AluOpType.is_gt
    is_equal = mybir.AluOpType.is_equal
    amin = mybir.AluOpType.min
    add = mybir.AluOpType.add

    nc = bass.Bass("TRN2", target_bir_lowering=False, debug=False,
                   enable_asserts=False, num_devices=N_CORES)
    x_in = nc.dram_tensor("x", [P, COLS], f32, kind="ExternalInput")
    y_out = nc.dram_tensor("y", [P, XCOLS], f32, kind="ExternalOutput")

    with contextlib.ExitStack() as stack:
        sem = lambda name: stack.enter_context(nc.semaphore(name))
        x_sem = sem("x_in")
        g_sem = sem("g")
        so_sem = sem("so")

        sb = stack.enter_context
        prod = sb(nc.sbuf_tensor("prod", [1, 48], f32))
        s3 = sb(nc.sbuf_tensor("s3", [1, 3], f32))
        smin = sb(nc.sbuf_tensor("smin", [1, 1], f32))
        gg = sb(nc.sbuf_tensor("gg", [1, 1], i32))
        xt = sb(nc.sbuf_tensor("xt", [P, COLS], f32))
        zt = sb(nc.sbuf_tensor("zt", [P, XCOLS], f32))

        # prod viewed as [1, 3, 16] so tensor_reduce(X) sums each layer's
        # 16 products into one dot
        pb = prod[:]
        prod3 = bass.AP(tensor=pb.tensor, offset=pb.offset,
                        ap=[list(pb.ap[0]), [16, 3], [1, 16]])

        with nc.Block() as block:

            @block.sync
            def _(sync):
                sync.dma_start(out=xt[:], in_=x_in.ap()).then_inc(x_sem, 16)
                sync.wait_ge(g_sem, 1)
                greg = sync.alloc_register("greg")
                sync.reg_load(greg, gg[0:1, 0:1])
                # snap declares the [0,1] range without emitting a SeqAssert
                # (walrus cannot encode SeqAssert)
                gval = sync.snap(greg, donate=True, min_val=0, max_val=1)
                sync.dma_start(out=y_out.ap(), in_=xt[:, 0:XCOLS],
                               cond=gval).then_inc(so_sem, 16)
                sync.wait_ge(so_sem, 32)

            @block.scalar
            def _(scalar):
                scalar.wait_ge(g_sem, 1)
                gnreg = scalar.alloc_register("gnreg")
                scalar.reg_load(gnreg, gg[0:1, 0:1])
                scalar.reg_alu(gnreg, gnreg, 0, is_equal)   # not g
                gnval = scalar.snap(gnreg, donate=True, min_val=0, max_val=1)
                scalar.dma_start(out=y_out.ap(), in_=zt[:], cond=gnval
                                 ).then_inc(so_sem, 16)

            @block.vector
            def _(vector):
                vector.memset(zt[:], 0)
                vector.wait_ge(x_sem, 16)
                bcv = xt[0:1, XCOLS:COLS]
                vector.tensor_mul(prod[:], bcv[:, 0:48], bcv[:, 48:96])
                vector.drain()
                vector.tensor_reduce(s3[:], prod3, axis=mybir.AxisListType.X,
                                     op=add)
                vector.drain()
                vector.tensor_reduce(smin[:], s3[:], axis=mybir.AxisListType.X,
                                     op=amin)
                vector.drain()
                vector.tensor_scalar(out=gg[:], in0=smin[:],
                                     scalar1=0.0, scalar2=None, op0=is_gt)
                vector.drain()
                vector.sem_inc(g_sem, 1)

    return nc


def _get_nc():
    if "nc" not in _cache:
        _cache["nc"] = _build()
    return _cache["nc"]


def _prep_in_maps(
    incoming_spikes,
    B_sensory, C_sensory, B_association, C_association,
    B_executive, C_executive,
):
    x = np.asarray(incoming_spikes)
    # spikes are {0,1}; pack them 1 bit each ((x>0) matches the
    # sigmoid(y)>0.5 threshold for any non-negative input)
    xb = np.packbits(np.asarray(x > 0), axis=1)          # [4096, 128] u8
    xw = xb.reshape(N_CORES, P, XCOLS * 4).view(np.float32)  # [8, 64, 256]
    bc = np.concatenate(
        [
            np.asarray(B_sensory, dtype=np.float32).reshape(16),
            np.asarray(B_association, dtype=np.float32).reshape(16),
            np.asarray(B_executive, dtype=np.float32).reshape(16),
            np.asarray(C_sensory, dtype=np.float32).reshape(16),
            np.asarray(C_association, dtype=np.float32).reshape(16),
            np.asarray(C_executive, dtype=np.float32).reshape(16),
        ]
    )
    slabs = []
    for i in range(N_CORES):
        slab = np.zeros((P, COLS), dtype=np.float32)
        slab[:, :XCOLS] = xw[i]
        slab[0, XCOLS:] = bc
        slabs.append(slab)
    return [{"x": slabs[i]} for i in range(N_CORES)]


def kernel(
    incoming_spikes,
    A_sensory, B_sensory, C_sensory,
    A_association, B_association, C_association,
    A_executive, B_executive, C_executive,
):
    from concourse.bass_utils import run_bass_kernel_spmd

    nc = _get_nc()
    in_maps = _prep_in_maps(
        incoming_spikes,
        B_sensory, C_sensory, B_association, C_association,
        B_executive, C_executive,
    )
    res = run_bass_kernel_spmd(nc, in_maps, list(range(N_CORES)))
    packed = np.concatenate(
        [
            np.ascontiguousarray(res.results[i]["y"])
            .view(np.uint8)
            .reshape(ROWS, WIDTH // 8)
            for i in range(N_CORES)
        ],
        axis=0,
    )
    return np.unpackbits(packed, axis=1).astype(np.float32)
